# revision 25
# baseline (speedup 1.0000x reference)
"""Trainium2 Bass kernel for dilated local attention.

Problem: q,k,v [B=8, d=768, N=6144] fp32; head_dim=32, kernel_size=3.
Per (batch, head, window) a 3x3 attention over 32-dim head vectors, where
window g groups tokens {g, g+2048, g+4096}.  Output [B, N, d] with token
n = 3*g + i and channel c = 32*h + cc.

Sharding: batch b -> core b (8 NeuronCores, no communication).

Per-core dataflow (natural layout, partitions = 128 channel rows = 4 heads):
  - cast-DMA loads q,k,v tiles [128c, 3i, 512g] fp32->bf16
  - DVE: tmp_ij = q_i * k_j elementwise (9 per tile)
  - PE:  9 accumulating matmuls with 0/1 mask weights -> S[36, 512] PSUM
         (mask sums each 32-row head segment and routes (i,j,h) to row
          12j+4i+h; accumulation over the 9 matmuls stacks all planes
          into one PSUM bank)
  - ACT: E = exp(scale*S) -> SBUF fp32
  - PE:  D[4i+h] = sum_j E[12j+4i+h] via ones-matmul -> PSUM
  - DVE: Dinv = reciprocal_approx_fast(D);  P = E * Dinv (bf16)
  - PE:  9 selector matmuls broadcast P rows back to 128 channel rows
  - ACT: copy PSUM -> SBUF bf16
  - DVE: tmp4_ij = Pbr_ij * v_j
  - PE:  matmul(lhsT=tmp4 chunk, rhs=I128) accumulated over j = transposed
         output u_i^T [g, c] in PSUM
  - ACT: copy PSUM -> SBUF fp32 assembling [128g, 768c] store tiles
  - HWDGE store to out rows n=3g+i (768*4B contiguous per row)
"""

import os
import sys

if "/opt/trn_rl_repo" not in sys.path:
    sys.path.insert(0, "/opt/trn_rl_repo")

from contextlib import ExitStack

import numpy as np

import concourse.bacc as bacc
import concourse.tile as tile
from concourse import mybir
from concourse.bass_utils import run_bass_kernel_spmd

B, D, N = 8, 768, 6144
HD, KS = 32, 3
H = D // HD  # 24 heads
G = N // KS  # 2048 windows
NCORES = 8
SCALE = float(HD) ** -0.5

CB = 6  # channel blocks of 128 (4 heads each)
F = 512  # windows per tile
FH = 512  # PSUM-bank-sized piece of F
NH = F // FH
GC = G // F  # g-chunks
GS = F // 128  # 128-wide subchunks per g-chunk

F32 = mybir.dt.float32
BF16 = mybir.dt.bfloat16

_CACHE: dict = {}


def _host_masks():
    """Constant 0/1 matrices used as PE weights (host side, fp32)."""
    # scores: out[m=32j+4i+h, f] += sum_{p in head h} tmp_ij[p, f]
    # (j planes start at 32-aligned partitions; rows 12..31 of each plane
    #  stay zero)
    wsc = np.zeros((KS * KS, 128, 96), np.float32)
    # broadcast: out[m, f] = P[32j+4i+m//32, f]
    wbr = np.zeros((KS * KS, 96, 128), np.float32)
    for i in range(KS):
        for j in range(KS):
            ij = i * KS + j
            for p in range(128):
                wsc[ij, p, 32 * j + 4 * i + p // 32] = 1.0
            for m in range(128):
                wbr[ij, 32 * j + 4 * i + m // 32, m] = 1.0
    # D replicated into every 32-aligned block:
    #   out[32jj + r, f] = sum_j E[32j + r, f]   for r < 12, any jj
    # unused rows (r >= 12) get D = E[row 12] = exp(0) = 1 so the
    # reciprocal stays finite.
    wd = np.zeros((96, 96), np.float32)
    for r in range(12):
        for j in range(KS):
            for jj in range(KS):
                wd[32 * j + r, 32 * jj + r] = 1.0
    for m in range(96):
        if m % 32 >= 12:
            wd[12, m] = 1.0
    ident = np.eye(128, dtype=np.float32)
    return wsc, wbr, wd, ident


def _build_kernel(ctx: ExitStack, tc: tile.TileContext, q, k, v, out, wsc, wbr, wd, ident):
    nc = tc.nc

    consts = ctx.enter_context(tc.tile_pool(name="consts", bufs=1))
    qkv_pool = ctx.enter_context(tc.tile_pool(name="qkv", bufs=6))
    tmp_pool = ctx.enter_context(tc.tile_pool(name="tmp", bufs=12))
    small_pool = ctx.enter_context(tc.tile_pool(name="small", bufs=4))
    brsb_pool = ctx.enter_context(tc.tile_pool(name="brsb", bufs=8))
    t4_pool = ctx.enter_context(tc.tile_pool(name="t4", bufs=9))
    out_pool = ctx.enter_context(tc.tile_pool(name="outsb", bufs=2))
    ps_s = ctx.enter_context(tc.tile_pool(name="psS", bufs=1, space="PSUM"))
    ps_d = ctx.enter_context(tc.tile_pool(name="psD", bufs=1, space="PSUM"))
    ps_br = ctx.enter_context(tc.tile_pool(name="psBr", bufs=3, space="PSUM"))
    ps_t = ctx.enter_context(tc.tile_pool(name="psT", bufs=3, space="PSUM"))

    # constant weights -> SBUF (bf16 for bf16 matmuls, fp32 for the D matmul)
    wsc_sb = consts.tile([128, KS * KS, 96], BF16)
    nc.gpsimd.dma_start(out=wsc_sb, in_=wsc[:, :, :].rearrange("n p f -> p n f"))
    wbr_sb = consts.tile([96, KS * KS, 128], BF16)
    nc.gpsimd.dma_start(out=wbr_sb, in_=wbr[:, :, :].rearrange("n p f -> p n f"))
    wd_sb = consts.tile([96, 96], F32)
    nc.gpsimd.dma_start(out=wd_sb, in_=wd[:, :])
    id_sb = consts.tile([128, 128], BF16)
    nc.gpsimd.dma_start(out=id_sb, in_=ident[:, :])

    # out viewed as [t, g, d]  (token n = 3g + t, t = h//8)
    out_r = out[:, :].rearrange("(g t) d -> t g d", t=KS)

    for gc in range(GC):
        # store tiles [128 g-rows, (gs, 768 c)] per t = h//8;
        # out channel layout within a row: c = 96*(h%8) + 32*i + cc
        osb = [
            out_pool.tile([128, GS * D], F32, tag=f"osb_{t}", name=f"osb_{gc}_{t}")
            for t in range(KS)
        ]
        n0 = gc * F
        for cb in range(CB):
            c0 = cb * 128
            qt = qkv_pool.tile([128, KS, F], BF16, tag="qt", name=f"qt_{gc}_{cb}")
            kt = qkv_pool.tile([128, KS, F], BF16, tag="kt", name=f"kt_{gc}_{cb}")
            vt = qkv_pool.tile([128, KS, F], BF16, tag="vt", name=f"vt_{gc}_{cb}")
            for srct, dst in ((q, qt), (k, kt), (v, vt)):
                nc.gpsimd.dma_start(
                    out=dst,
                    in_=srct[c0 : c0 + 128, :]
                    .rearrange("p (i g) -> p i g", i=KS)[:, :, n0 : n0 + F],
                )

            # tmp_ij = q_i * k_j  (bf16, DVE 2x)
            tmp = []
            for i in range(KS):
                for j in range(KS):
                    t = tmp_pool.tile(
                        [128, F], BF16, tag="tmp", name=f"tmp_{gc}_{cb}_{i}{j}"
                    )
                    nc.vector.tensor_mul(out=t, in0=qt[:, i, :], in1=kt[:, j, :])
                    tmp.append(t)

            # scores: accumulating matmuls -> S[96, F] (rows 32j+4i+h),
            # one 9-matmul accumulation group per 512-wide half
            s_ps = ps_s.tile([96, F], F32, tag="S", name=f"S_{gc}_{cb}")
            for half in range(NH):
                h0 = half * FH
                for ij in range(KS * KS):
                    nc.tensor.matmul(
                        s_ps[:, h0 : h0 + FH],
                        lhsT=wsc_sb[:, ij, :],
                        rhs=tmp[ij][:, h0 : h0 + FH],
                        start=(ij == 0),
                        stop=(ij == KS * KS - 1),
                    )

            # E = exp(scale * S)
            e_sb = small_pool.tile([96, F], F32, tag="E", name=f"E_{gc}_{cb}")
            nc.scalar.activation(
                out=e_sb, in_=s_ps, func=mybir.ActivationFunctionType.Exp, scale=SCALE
            )

            # D (replicated to the three 32-blocks);  Dinv;  P = E * Dinv
            dinv = small_pool.tile([96, F], F32, tag="Dinv", name=f"Di_{gc}_{cb}")
            for half in range(NH):
                h0 = half * FH
                d_ps = ps_d.tile(
                    [96, FH], F32, tag="D", name=f"D_{gc}_{cb}_{half}"
                )
                nc.tensor.matmul(
                    d_ps, lhsT=wd_sb, rhs=e_sb[:, h0 : h0 + FH],
                    start=True, stop=True,
                )
                nc.vector.reciprocal_approx_fast(
                    out=dinv[:, h0 : h0 + FH], in_=d_ps
                )
            p_sb = small_pool.tile([96, F], BF16, tag="P", name=f"P_{gc}_{cb}")
            nc.vector.tensor_mul(out=p_sb, in0=e_sb, in1=dinv)

            # Softmax weights sum to 1, so
            #   u_i = v_1 + P_i0 (v_0 - v_1) + P_i2 (v_2 - v_1)
            # -> only the j=0 and j=2 probability planes need broadcasting;
            #    v_1 enters the transpose-accumulation directly.
            dv0 = t4_pool.tile([128, F], BF16, tag="dv0", name=f"dv0_{gc}_{cb}", bufs=3)
            dv2 = t4_pool.tile([128, F], BF16, tag="dv2", name=f"dv2_{gc}_{cb}", bufs=3)
            nc.vector.tensor_sub(out=dv0, in0=vt[:, 0, :], in1=vt[:, 1, :])
            nc.vector.tensor_sub(out=dv2, in0=vt[:, 2, :], in1=vt[:, 1, :])
            t4 = {}
            for i in range(KS):
                for j, dv in ((0, dv0), (2, dv2)):
                    ij = i * KS + j
                    br_sb = brsb_pool.tile(
                        [128, F], BF16, tag="brsb", name=f"brsb_{gc}_{cb}_{i}{j}"
                    )
                    for half in range(NH):
                        h0 = half * FH
                        br_ps = ps_br.tile(
                            [128, FH], F32, tag="Br",
                            name=f"brp_{gc}_{cb}_{i}{j}_{half}",
                        )
                        nc.tensor.matmul(
                            br_ps, lhsT=wbr_sb[:, ij, :],
                            rhs=p_sb[:, h0 : h0 + FH], start=True, stop=True,
                        )
                        nc.scalar.copy(out=br_sb[:, h0 : h0 + FH], in_=br_ps)
                    t = t4_pool.tile(
                        [128, F], BF16, tag="t4", name=f"t4_{gc}_{cb}_{i}{j}"
                    )
                    eng = nc.gpsimd if (i, j) in ((0, 2), (2, 0)) else nc.vector
                    eng.tensor_mul(out=t, in0=br_sb, in1=dv)
                    t4[(i, j)] = t

            # transpose + accumulate:  u_i^T [g, c] in PSUM
            for i in range(KS):
                t_ps = ps_t.tile([128, F], F32, tag="T", name=f"T_{gc}_{cb}_{i}")
                for gs in range(GS):
                    sl = slice(gs * 128, (gs + 1) * 128)
                    for step, lhs in enumerate(
                        (t4[(i, 0)][:, sl], t4[(i, 2)][:, sl], vt[:, 1, sl])
                    ):
                        nc.tensor.matmul(
                            t_ps[:, sl],
                            lhsT=lhs,
                            rhs=id_sb,
                            start=(step == 0),
                            stop=(step == 2),
                        )
                # route the 4 local heads (hl) to output columns
                #   384*(cb%2) + 96*hl + 32*i + cc  in store tile t=cb//2,
                # one strided copy covering all 8 g-subchunks
                tt = cb // 2
                dst = osb[tt].rearrange(
                    "p (gs hl i cc) -> p gs hl i cc", gs=GS, hl=8, i=KS
                )[:, :, 4 * (cb % 2) : 4 * (cb % 2) + 4, i, :]
                src_ap = t_ps.rearrange("p (gs hl cc) -> p gs hl cc", gs=GS, hl=4)
                nc.scalar.copy(out=dst, in_=src_ap)

        for t in range(KS):
            for gs in range(GS):
                g0 = gc * F + gs * 128
                nc.sync.dma_start(
                    out=out_r[t, g0 : g0 + 128, :],
                    in_=osb[t][:, gs * D : (gs + 1) * D],
                )


def _get_nc():
    if "nc" in _CACHE:
        return _CACHE["nc"]
    nc = bacc.Bacc("TRN2", target_bir_lowering=False, debug=False, num_devices=NCORES)
    q = nc.dram_tensor("q", [D, N], F32, kind="ExternalInput").ap()
    k = nc.dram_tensor("k", [D, N], F32, kind="ExternalInput").ap()
    v = nc.dram_tensor("v", [D, N], F32, kind="ExternalInput").ap()
    out = nc.dram_tensor("out", [N, D], F32, kind="ExternalOutput").ap()
    wsc = nc.dram_tensor("wsc", [KS * KS, 128, 96], BF16, kind="ExternalInput").ap()
    wbr = nc.dram_tensor("wbr", [KS * KS, 96, 128], BF16, kind="ExternalInput").ap()
    wd = nc.dram_tensor("wd", [96, 96], F32, kind="ExternalInput").ap()
    ident = nc.dram_tensor("ident", [128, 128], BF16, kind="ExternalInput").ap()
    with tile.TileContext(nc) as tc:
        with ExitStack() as ctx:
            _build_kernel(ctx, tc, q, k, v, out, wsc, wbr, wd, ident)
    nc.compile()
    _CACHE["nc"] = nc
    return nc


def kernel(q, k, v, head_dim, kernel_size, _trace=False, _trace_kwargs=None):
    assert int(head_dim) == HD and int(kernel_size) == KS
    q = np.asarray(q, dtype=np.float32)
    k = np.asarray(k, dtype=np.float32)
    v = np.asarray(v, dtype=np.float32)
    assert q.shape == (B, D, N)

    nc = _get_nc()
    bf = mybir.dt.np(BF16)
    wsc, wbr, wd, ident = _host_masks()
    consts = {
        "wsc": wsc.astype(bf),
        "wbr": wbr.astype(bf),
        "wd": wd,
        "ident": ident.astype(bf),
    }
    in_maps = [
        {"q": q[b], "k": k[b], "v": v[b], **consts} for b in range(B)
    ]
    res = run_bass_kernel_spmd(
        nc,
        in_maps,
        core_ids=list(range(NCORES)),
        trace=_trace,
        **(_trace_kwargs or {}),
    )
    out = np.stack([res.results[b]["out"] for b in range(B)], axis=0)
    _CACHE["last_results"] = res
    return out


if __name__ == "__main__":
    rng = np.random.default_rng(0)
    qq = rng.standard_normal((B, D, N), dtype=np.float32)
    kk = rng.standard_normal((B, D, N), dtype=np.float32)
    vv = rng.standard_normal((B, D, N), dtype=np.float32)
    o = kernel(qq, kk, vv, HD, KS)
    print("out", o.shape, o.dtype, float(np.abs(o).max()))


# revision 30
# speedup vs baseline: 1.1590x; 1.1590x over previous
"""Trainium2 Bass kernel for dilated local attention.

Problem: q,k,v [B=8, d=768, N=6144] fp32; head_dim=32, kernel_size=3.
Per (batch, head, window) a 3x3 attention over 32-dim head vectors, where
window g groups tokens {g, g+2048, g+4096}.  Output [B, N, d] with token
n = 3*g + i and channel c = 32*h + cc.

Sharding: batch b -> core b (8 NeuronCores, no communication).

Per-core dataflow (natural layout, partitions = 128 channel rows = 4 heads):
  - cast-DMA loads q,k,v tiles [128c, 3i, 512g] fp32->bf16
  - DVE: tmp_ij = q_i * k_j elementwise (9 per tile)
  - PE:  9 accumulating matmuls with 0/1 mask weights -> S[36, 512] PSUM
         (mask sums each 32-row head segment and routes (i,j,h) to row
          12j+4i+h; accumulation over the 9 matmuls stacks all planes
          into one PSUM bank)
  - ACT: E = exp(scale*S) -> SBUF fp32
  - PE:  D[4i+h] = sum_j E[12j+4i+h] via ones-matmul -> PSUM
  - DVE: Dinv = reciprocal_approx_fast(D);  P = E * Dinv (bf16)
  - PE:  9 selector matmuls broadcast P rows back to 128 channel rows
  - ACT: copy PSUM -> SBUF bf16
  - DVE: tmp4_ij = Pbr_ij * v_j
  - PE:  matmul(lhsT=tmp4 chunk, rhs=I128) accumulated over j = transposed
         output u_i^T [g, c] in PSUM
  - ACT: copy PSUM -> SBUF fp32 assembling [128g, 768c] store tiles
  - HWDGE store to out rows n=3g+i (768*4B contiguous per row)
"""

import os
import sys

if "/opt/trn_rl_repo" not in sys.path:
    sys.path.insert(0, "/opt/trn_rl_repo")

from contextlib import ExitStack

import numpy as np

import concourse.bacc as bacc
import concourse.tile as tile
from concourse import mybir
from concourse.bass_utils import run_bass_kernel_spmd

B, D, N = 8, 768, 6144
HD, KS = 32, 3
H = D // HD  # 24 heads
G = N // KS  # 2048 windows
NCORES = 8
SCALE = float(HD) ** -0.5

CB = 6  # channel blocks of 128 (4 heads each)
F = 512  # windows per tile
FH = 512  # PSUM-bank-sized piece of F
NH = F // FH
GC = G // F  # g-chunks
GS = F // 128  # 128-wide subchunks per g-chunk

F32 = mybir.dt.float32
BF16 = mybir.dt.bfloat16

_CACHE: dict = {}


def _host_masks():
    """Constant 0/1 matrices used as PE weights (host side, fp32)."""
    # scores: out[m=32j+4i+h, f] += sum_{p in head h} tmp_ij[p, f]
    # (j planes start at 32-aligned partitions; rows 12..31 of each plane
    #  stay zero)
    wsc = np.zeros((KS * KS, 128, 96), np.float32)
    # broadcast: out[m, f] = P[32j+4i+m//32, f]
    wbr = np.zeros((KS * KS, 96, 128), np.float32)
    for i in range(KS):
        for j in range(KS):
            ij = i * KS + j
            for p in range(128):
                wsc[ij, p, 32 * j + 4 * i + p // 32] = 1.0
            for m in range(128):
                wbr[ij, 32 * j + 4 * i + m // 32, m] = 1.0
    # D replicated into every 32-aligned block:
    #   out[32jj + r, f] = sum_j E[32j + r, f]   for r < 12, any jj
    # unused rows (r >= 12) get D = E[row 12] = exp(0) = 1 so the
    # reciprocal stays finite.
    wd = np.zeros((96, 96), np.float32)
    for r in range(12):
        for j in range(KS):
            for jj in range(KS):
                wd[32 * j + r, 32 * jj + r] = 1.0
    for m in range(96):
        if m % 32 >= 12:
            wd[12, m] = 1.0
    ident = np.eye(128, dtype=np.float32)
    return wsc, wbr, wd, ident


def _build_kernel(ctx: ExitStack, tc: tile.TileContext, q, k, v, out, wsc, wbr, wd, ident):
    nc = tc.nc

    consts = ctx.enter_context(tc.tile_pool(name="consts", bufs=1))
    qkv_pool = ctx.enter_context(tc.tile_pool(name="qkv", bufs=6))
    tmp_pool = ctx.enter_context(tc.tile_pool(name="tmp", bufs=12))
    small_pool = ctx.enter_context(tc.tile_pool(name="small", bufs=4))
    brsb_pool = ctx.enter_context(tc.tile_pool(name="brsb", bufs=8))
    t4_pool = ctx.enter_context(tc.tile_pool(name="t4", bufs=9))
    out_pool = ctx.enter_context(tc.tile_pool(name="outsb", bufs=2))
    ps_s = ctx.enter_context(tc.tile_pool(name="psS", bufs=1, space="PSUM"))
    ps_d = ctx.enter_context(tc.tile_pool(name="psD", bufs=1, space="PSUM"))
    ps_br = ctx.enter_context(tc.tile_pool(name="psBr", bufs=3, space="PSUM"))
    ps_t = ctx.enter_context(tc.tile_pool(name="psT", bufs=3, space="PSUM"))

    # constant weights -> SBUF (bf16 for bf16 matmuls, fp32 for the D matmul)
    wsc_sb = consts.tile([128, KS * KS, 96], BF16)
    nc.gpsimd.dma_start(out=wsc_sb, in_=wsc[:, :, :].rearrange("n p f -> p n f"))
    wbr_sb = consts.tile([96, KS * KS, 128], BF16)
    nc.gpsimd.dma_start(out=wbr_sb, in_=wbr[:, :, :].rearrange("n p f -> p n f"))
    wd_sb = consts.tile([96, 96], F32)
    nc.gpsimd.dma_start(out=wd_sb, in_=wd[:, :])
    id_sb = consts.tile([128, 128], BF16)
    nc.gpsimd.dma_start(out=id_sb, in_=ident[:, :])

    # out viewed as [t, g, d]  (token n = 3g + t, t = h//8)
    out_r = out[:, :].rearrange("(g t) d -> t g d", t=KS)

    def phase1(blk):
        gc, cb = blk
        n0 = gc * F
        c0 = cb * 128
        qt = qkv_pool.tile([128, KS, F], BF16, tag="qt", name=f"qt_{gc}_{cb}")
        kt = qkv_pool.tile([128, KS, F], BF16, tag="kt", name=f"kt_{gc}_{cb}")
        vt = qkv_pool.tile([128, KS, F], BF16, tag="vt", name=f"vt_{gc}_{cb}")
        for srct, dst in ((q, qt), (k, kt), (v, vt)):
            nc.gpsimd.dma_start(
                out=dst,
                in_=srct[c0 : c0 + 128, :]
                .rearrange("p (i g) -> p i g", i=KS)[:, :, n0 : n0 + F],
            )

        # tmp_ij = q_i * k_j  (bf16, DVE 2x)
        tmp = []
        for i in range(KS):
            for j in range(KS):
                t = tmp_pool.tile(
                    [128, F], BF16, tag="tmp", name=f"tmp_{gc}_{cb}_{i}{j}"
                )
                nc.vector.tensor_mul(out=t, in0=qt[:, i, :], in1=kt[:, j, :])
                tmp.append(t)

        # scores: accumulating matmuls -> S[96, F] (rows 32j+4i+h)
        s_ps = ps_s.tile([96, F], F32, tag="S", name=f"S_{gc}_{cb}")
        for half in range(NH):
            h0 = half * FH
            for ij in range(KS * KS):
                nc.tensor.matmul(
                    s_ps[:, h0 : h0 + FH],
                    lhsT=wsc_sb[:, ij, :],
                    rhs=tmp[ij][:, h0 : h0 + FH],
                    start=(ij == 0),
                    stop=(ij == KS * KS - 1),
                )

        # E = exp(scale * S)
        e_sb = small_pool.tile([96, F], F32, tag="E", name=f"E_{gc}_{cb}")
        nc.scalar.activation(
            out=e_sb, in_=s_ps, func=mybir.ActivationFunctionType.Exp, scale=SCALE
        )

        # D (replicated to the three 32-blocks);  Dinv;  P = E * Dinv
        dinv = small_pool.tile([96, F], F32, tag="Dinv", name=f"Di_{gc}_{cb}")
        for half in range(NH):
            h0 = half * FH
            d_ps = ps_d.tile([96, FH], F32, tag="D", name=f"D_{gc}_{cb}_{half}")
            nc.tensor.matmul(
                d_ps, lhsT=wd_sb, rhs=e_sb[:, h0 : h0 + FH], start=True, stop=True
            )
            nc.vector.reciprocal_approx_fast(out=dinv[:, h0 : h0 + FH], in_=d_ps)
        p_sb = small_pool.tile([96, F], BF16, tag="P", name=f"P_{gc}_{cb}")
        nc.vector.tensor_mul(out=p_sb, in0=e_sb, in1=dinv)
        return {"vt": vt, "p_sb": p_sb}

    def phase2(blk, st, osb):
        gc, cb = blk
        vt, p_sb = st["vt"], st["p_sb"]
        # Softmax weights sum to 1, so
        #   u_i = v_1 + P_i0 (v_0 - v_1) + P_i2 (v_2 - v_1)
        dv0 = t4_pool.tile([128, F], BF16, tag="dv0", name=f"dv0_{gc}_{cb}", bufs=3)
        dv2 = t4_pool.tile([128, F], BF16, tag="dv2", name=f"dv2_{gc}_{cb}", bufs=3)
        nc.vector.tensor_sub(out=dv0, in0=vt[:, 0, :], in1=vt[:, 1, :])
        nc.vector.tensor_sub(out=dv2, in0=vt[:, 2, :], in1=vt[:, 1, :])
        t4 = {}
        for i in range(KS):
            for j, dv in ((0, dv0), (2, dv2)):
                ij = i * KS + j
                br_sb = brsb_pool.tile(
                    [128, F], BF16, tag="brsb", name=f"brsb_{gc}_{cb}_{i}{j}"
                )
                for half in range(NH):
                    h0 = half * FH
                    br_ps = ps_br.tile(
                        [128, FH], F32, tag="Br", name=f"brp_{gc}_{cb}_{i}{j}_{half}"
                    )
                    nc.tensor.matmul(
                        br_ps, lhsT=wbr_sb[:, ij, :],
                        rhs=p_sb[:, h0 : h0 + FH], start=True, stop=True,
                    )
                    nc.scalar.copy(out=br_sb[:, h0 : h0 + FH], in_=br_ps)
                t = t4_pool.tile(
                    [128, F], BF16, tag="t4", name=f"t4_{gc}_{cb}_{i}{j}"
                )
                eng = nc.gpsimd if (i, j) in ((0, 2), (2, 0)) else nc.vector
                eng.tensor_mul(out=t, in0=br_sb, in1=dv)
                t4[(i, j)] = t

        # transpose + accumulate:  u_i^T [g, c] in PSUM
        for i in range(KS):
            t_ps = ps_t.tile([128, F], F32, tag="T", name=f"T_{gc}_{cb}_{i}")
            for gs in range(GS):
                sl = slice(gs * 128, (gs + 1) * 128)
                for step, lhs in enumerate(
                    (t4[(i, 0)][:, sl], t4[(i, 2)][:, sl], vt[:, 1, sl])
                ):
                    nc.tensor.matmul(
                        t_ps[:, sl], lhsT=lhs, rhs=id_sb,
                        start=(step == 0), stop=(step == 2),
                    )
            # route the 4 local heads (hl) to output columns
            #   384*(cb%2) + 96*hl + 32*i + cc  in store tile t=cb//2
            tt = cb // 2
            dst = osb[tt].rearrange(
                "p (gs hl i cc) -> p gs hl i cc", gs=GS, hl=8, i=KS
            )[:, :, 4 * (cb % 2) : 4 * (cb % 2) + 4, i, :]
            src_ap = t_ps.rearrange("p (gs hl cc) -> p gs hl cc", gs=GS, hl=4)
            nc.scalar.copy(out=dst, in_=src_ap)

    def stores(gc, osb):
        for t in range(KS):
            for gs in range(GS):
                g0 = gc * F + gs * 128
                nc.sync.dma_start(
                    out=out_r[t, g0 : g0 + 128, :],
                    in_=osb[t][:, gs * D : (gs + 1) * D],
                )

    # software-pipelined emission: phase1(n+1) before phase2(n)
    blocks = [(gc, cb) for gc in range(GC) for cb in range(CB)]
    osb_by_gc = {}
    for gc in range(GC):
        osb_by_gc[gc] = [
            out_pool.tile([128, GS * D], F32, tag=f"osb_{t}", name=f"osb_{gc}_{t}")
            for t in range(KS)
        ]
    state = {}
    state[blocks[0]] = phase1(blocks[0])
    for idx, blk in enumerate(blocks):
        if idx + 1 < len(blocks):
            state[blocks[idx + 1]] = phase1(blocks[idx + 1])
        phase2(blk, state.pop(blk), osb_by_gc[blk[0]])
        if blk[1] == CB - 1:
            stores(blk[0], osb_by_gc[blk[0]])


def _get_nc():
    if "nc" in _CACHE:
        return _CACHE["nc"]
    nc = bacc.Bacc("TRN2", target_bir_lowering=False, debug=False, num_devices=NCORES)
    q = nc.dram_tensor("q", [D, N], F32, kind="ExternalInput").ap()
    k = nc.dram_tensor("k", [D, N], F32, kind="ExternalInput").ap()
    v = nc.dram_tensor("v", [D, N], F32, kind="ExternalInput").ap()
    out = nc.dram_tensor("out", [N, D], F32, kind="ExternalOutput").ap()
    wsc = nc.dram_tensor("wsc", [KS * KS, 128, 96], BF16, kind="ExternalInput").ap()
    wbr = nc.dram_tensor("wbr", [KS * KS, 96, 128], BF16, kind="ExternalInput").ap()
    wd = nc.dram_tensor("wd", [96, 96], F32, kind="ExternalInput").ap()
    ident = nc.dram_tensor("ident", [128, 128], BF16, kind="ExternalInput").ap()
    with tile.TileContext(nc) as tc:
        with ExitStack() as ctx:
            _build_kernel(ctx, tc, q, k, v, out, wsc, wbr, wd, ident)
    nc.compile()
    _CACHE["nc"] = nc
    return nc


def kernel(q, k, v, head_dim, kernel_size, _trace=False, _trace_kwargs=None):
    assert int(head_dim) == HD and int(kernel_size) == KS
    q = np.asarray(q, dtype=np.float32)
    k = np.asarray(k, dtype=np.float32)
    v = np.asarray(v, dtype=np.float32)
    assert q.shape == (B, D, N)

    nc = _get_nc()
    bf = mybir.dt.np(BF16)
    wsc, wbr, wd, ident = _host_masks()
    consts = {
        "wsc": wsc.astype(bf),
        "wbr": wbr.astype(bf),
        "wd": wd,
        "ident": ident.astype(bf),
    }
    in_maps = [
        {"q": q[b], "k": k[b], "v": v[b], **consts} for b in range(B)
    ]
    res = run_bass_kernel_spmd(
        nc,
        in_maps,
        core_ids=list(range(NCORES)),
        trace=_trace,
        **(_trace_kwargs or {}),
    )
    out = np.stack([res.results[b]["out"] for b in range(B)], axis=0)
    _CACHE["last_results"] = res
    return out


if __name__ == "__main__":
    rng = np.random.default_rng(0)
    qq = rng.standard_normal((B, D, N), dtype=np.float32)
    kk = rng.standard_normal((B, D, N), dtype=np.float32)
    vv = rng.standard_normal((B, D, N), dtype=np.float32)
    o = kernel(qq, kk, vv, HD, KS)
    print("out", o.shape, o.dtype, float(np.abs(o).max()))


# revision 31
# speedup vs baseline: 1.1593x; 1.0002x over previous
"""Trainium2 Bass kernel for dilated local attention.

Problem: q,k,v [B=8, d=768, N=6144] fp32; head_dim=32, kernel_size=3.
Per (batch, head, window) a 3x3 attention over 32-dim head vectors, where
window g groups tokens {g, g+2048, g+4096}.  Output [B, N, d] with token
n = 3*g + i and channel c = 32*h + cc.

Sharding: batch b -> core b (8 NeuronCores, no communication).

Per-core dataflow (natural layout, partitions = 128 channel rows = 4 heads):
  - cast-DMA loads q,k,v tiles [128c, 3i, 512g] fp32->bf16
  - DVE: tmp_ij = q_i * k_j elementwise (9 per tile)
  - PE:  9 accumulating matmuls with 0/1 mask weights -> S[36, 512] PSUM
         (mask sums each 32-row head segment and routes (i,j,h) to row
          12j+4i+h; accumulation over the 9 matmuls stacks all planes
          into one PSUM bank)
  - ACT: E = exp(scale*S) -> SBUF fp32
  - PE:  D[4i+h] = sum_j E[12j+4i+h] via ones-matmul -> PSUM
  - DVE: Dinv = reciprocal_approx_fast(D);  P = E * Dinv (bf16)
  - PE:  9 selector matmuls broadcast P rows back to 128 channel rows
  - ACT: copy PSUM -> SBUF bf16
  - DVE: tmp4_ij = Pbr_ij * v_j
  - PE:  matmul(lhsT=tmp4 chunk, rhs=I128) accumulated over j = transposed
         output u_i^T [g, c] in PSUM
  - ACT: copy PSUM -> SBUF fp32 assembling [128g, 768c] store tiles
  - HWDGE store to out rows n=3g+i (768*4B contiguous per row)
"""

import os
import sys

if "/opt/trn_rl_repo" not in sys.path:
    sys.path.insert(0, "/opt/trn_rl_repo")

from contextlib import ExitStack

import numpy as np

import concourse.bacc as bacc
import concourse.tile as tile
from concourse import mybir
from concourse.bass_utils import run_bass_kernel_spmd

B, D, N = 8, 768, 6144
HD, KS = 32, 3
H = D // HD  # 24 heads
G = N // KS  # 2048 windows
NCORES = 8
SCALE = float(HD) ** -0.5

CB = 6  # channel blocks of 128 (4 heads each)
F = 512  # windows per tile
FH = 512  # PSUM-bank-sized piece of F
NH = F // FH
GC = G // F  # g-chunks
GS = F // 128  # 128-wide subchunks per g-chunk

F32 = mybir.dt.float32
BF16 = mybir.dt.bfloat16

_CACHE: dict = {}


def _host_masks():
    """Constant 0/1 matrices used as PE weights (host side, fp32)."""
    # scores: out[m=32j+4i+h, f] += sum_{p in head h} tmp_ij[p, f]
    # (j planes start at 32-aligned partitions; rows 12..31 of each plane
    #  stay zero)
    wsc = np.zeros((KS * KS, 128, 96), np.float32)
    # broadcast: out[m, f] = P[32j+4i+m//32, f]
    wbr = np.zeros((KS * KS, 96, 128), np.float32)
    for i in range(KS):
        for j in range(KS):
            ij = i * KS + j
            for p in range(128):
                wsc[ij, p, 32 * j + 4 * i + p // 32] = 1.0
            for m in range(128):
                wbr[ij, 32 * j + 4 * i + m // 32, m] = 1.0
    # D replicated into every 32-aligned block:
    #   out[32jj + r, f] = sum_j E[32j + r, f]   for r < 12, any jj
    # unused rows (r >= 12) get D = E[row 12] = exp(0) = 1 so the
    # reciprocal stays finite.
    wd = np.zeros((96, 96), np.float32)
    for r in range(12):
        for j in range(KS):
            for jj in range(KS):
                wd[32 * j + r, 32 * jj + r] = 1.0
    for m in range(96):
        if m % 32 >= 12:
            wd[12, m] = 1.0
    ident = np.eye(128, dtype=np.float32)
    return wsc, wbr, wd, ident


def _build_kernel(ctx: ExitStack, tc: tile.TileContext, q, k, v, out, wsc, wbr, wd, ident):
    nc = tc.nc

    consts = ctx.enter_context(tc.tile_pool(name="consts", bufs=1))
    qkv_pool = ctx.enter_context(tc.tile_pool(name="qkv", bufs=6))
    tmp_pool = ctx.enter_context(tc.tile_pool(name="tmp", bufs=12))
    small_pool = ctx.enter_context(tc.tile_pool(name="small", bufs=4))
    brsb_pool = ctx.enter_context(tc.tile_pool(name="brsb", bufs=10))
    t4_pool = ctx.enter_context(tc.tile_pool(name="t4", bufs=10))
    out_pool = ctx.enter_context(tc.tile_pool(name="outsb", bufs=2))
    ps_s = ctx.enter_context(tc.tile_pool(name="psS", bufs=1, space="PSUM"))
    ps_d = ctx.enter_context(tc.tile_pool(name="psD", bufs=1, space="PSUM"))
    ps_br = ctx.enter_context(tc.tile_pool(name="psBr", bufs=3, space="PSUM"))
    ps_t = ctx.enter_context(tc.tile_pool(name="psT", bufs=3, space="PSUM"))

    # constant weights -> SBUF (bf16 for bf16 matmuls, fp32 for the D matmul)
    wsc_sb = consts.tile([128, KS * KS, 96], BF16)
    nc.gpsimd.dma_start(out=wsc_sb, in_=wsc[:, :, :].rearrange("n p f -> p n f"))
    wbr_sb = consts.tile([96, KS * KS, 128], BF16)
    nc.gpsimd.dma_start(out=wbr_sb, in_=wbr[:, :, :].rearrange("n p f -> p n f"))
    wd_sb = consts.tile([96, 96], F32)
    nc.gpsimd.dma_start(out=wd_sb, in_=wd[:, :])
    id_sb = consts.tile([128, 128], BF16)
    nc.gpsimd.dma_start(out=id_sb, in_=ident[:, :])

    # out viewed as [t, g, d]  (token n = 3g + t, t = h//8)
    out_r = out[:, :].rearrange("(g t) d -> t g d", t=KS)

    def phase1(blk):
        gc, cb = blk
        n0 = gc * F
        c0 = cb * 128
        qt = qkv_pool.tile([128, KS, F], BF16, tag="qt", name=f"qt_{gc}_{cb}")
        kt = qkv_pool.tile([128, KS, F], BF16, tag="kt", name=f"kt_{gc}_{cb}")
        vt = qkv_pool.tile([128, KS, F], BF16, tag="vt", name=f"vt_{gc}_{cb}")
        for srct, dst in ((q, qt), (k, kt), (v, vt)):
            nc.gpsimd.dma_start(
                out=dst,
                in_=srct[c0 : c0 + 128, :]
                .rearrange("p (i g) -> p i g", i=KS)[:, :, n0 : n0 + F],
            )

        # tmp_ij = q_i * k_j  (bf16, DVE 2x)
        tmp = []
        for i in range(KS):
            for j in range(KS):
                t = tmp_pool.tile(
                    [128, F], BF16, tag="tmp", name=f"tmp_{gc}_{cb}_{i}{j}"
                )
                nc.vector.tensor_mul(out=t, in0=qt[:, i, :], in1=kt[:, j, :])
                tmp.append(t)

        # scores: accumulating matmuls -> S[96, F] (rows 32j+4i+h)
        s_ps = ps_s.tile([96, F], F32, tag="S", name=f"S_{gc}_{cb}")
        for half in range(NH):
            h0 = half * FH
            for ij in range(KS * KS):
                nc.tensor.matmul(
                    s_ps[:, h0 : h0 + FH],
                    lhsT=wsc_sb[:, ij, :],
                    rhs=tmp[ij][:, h0 : h0 + FH],
                    start=(ij == 0),
                    stop=(ij == KS * KS - 1),
                )

        # E = exp(scale * S)
        e_sb = small_pool.tile([96, F], F32, tag="E", name=f"E_{gc}_{cb}")
        nc.scalar.activation(
            out=e_sb, in_=s_ps, func=mybir.ActivationFunctionType.Exp, scale=SCALE
        )

        # D (replicated to the three 32-blocks);  Dinv;  P = E * Dinv
        dinv = small_pool.tile([96, F], F32, tag="Dinv", name=f"Di_{gc}_{cb}")
        for half in range(NH):
            h0 = half * FH
            d_ps = ps_d.tile([96, FH], F32, tag="D", name=f"D_{gc}_{cb}_{half}")
            nc.tensor.matmul(
                d_ps, lhsT=wd_sb, rhs=e_sb[:, h0 : h0 + FH], start=True, stop=True
            )
            nc.vector.reciprocal_approx_fast(out=dinv[:, h0 : h0 + FH], in_=d_ps)
        p_sb = small_pool.tile([96, F], BF16, tag="P", name=f"P_{gc}_{cb}")
        nc.vector.tensor_mul(out=p_sb, in0=e_sb, in1=dinv)
        return {"vt": vt, "p_sb": p_sb}

    def phase2(blk, st, osb):
        gc, cb = blk
        vt, p_sb = st["vt"], st["p_sb"]
        # Softmax weights sum to 1, so
        #   u_i = v_1 + P_i0 (v_0 - v_1) + P_i2 (v_2 - v_1)
        dv0 = t4_pool.tile([128, F], BF16, tag="dv0", name=f"dv0_{gc}_{cb}", bufs=3)
        dv2 = t4_pool.tile([128, F], BF16, tag="dv2", name=f"dv2_{gc}_{cb}", bufs=3)
        nc.vector.tensor_sub(out=dv0, in0=vt[:, 0, :], in1=vt[:, 1, :])
        nc.vector.tensor_sub(out=dv2, in0=vt[:, 2, :], in1=vt[:, 1, :])
        t4 = {}
        for i in range(KS):
            for j, dv in ((0, dv0), (2, dv2)):
                ij = i * KS + j
                br_sb = brsb_pool.tile(
                    [128, F], BF16, tag="brsb", name=f"brsb_{gc}_{cb}_{i}{j}"
                )
                for half in range(NH):
                    h0 = half * FH
                    br_ps = ps_br.tile(
                        [128, FH], F32, tag="Br", name=f"brp_{gc}_{cb}_{i}{j}_{half}"
                    )
                    nc.tensor.matmul(
                        br_ps, lhsT=wbr_sb[:, ij, :],
                        rhs=p_sb[:, h0 : h0 + FH], start=True, stop=True,
                    )
                    nc.scalar.copy(out=br_sb[:, h0 : h0 + FH], in_=br_ps)
                t = t4_pool.tile(
                    [128, F], BF16, tag="t4", name=f"t4_{gc}_{cb}_{i}{j}"
                )
                eng = nc.gpsimd if (i, j) in ((0, 2), (2, 0)) else nc.vector
                eng.tensor_mul(out=t, in0=br_sb, in1=dv)
                t4[(i, j)] = t

        # transpose + accumulate:  u_i^T [g, c] in PSUM
        for i in range(KS):
            t_ps = ps_t.tile([128, F], F32, tag="T", name=f"T_{gc}_{cb}_{i}")
            for gs in range(GS):
                sl = slice(gs * 128, (gs + 1) * 128)
                for step, lhs in enumerate(
                    (t4[(i, 0)][:, sl], t4[(i, 2)][:, sl], vt[:, 1, sl])
                ):
                    nc.tensor.matmul(
                        t_ps[:, sl], lhsT=lhs, rhs=id_sb,
                        start=(step == 0), stop=(step == 2),
                    )
            # route the 4 local heads (hl) to output columns
            #   384*(cb%2) + 96*hl + 32*i + cc  in store tile t=cb//2
            tt = cb // 2
            dst = osb[tt].rearrange(
                "p (gs hl i cc) -> p gs hl i cc", gs=GS, hl=8, i=KS
            )[:, :, 4 * (cb % 2) : 4 * (cb % 2) + 4, i, :]
            src_ap = t_ps.rearrange("p (gs hl cc) -> p gs hl cc", gs=GS, hl=4)
            nc.scalar.copy(out=dst, in_=src_ap)

    def stores(gc, osb):
        for t in range(KS):
            for gs in range(GS):
                g0 = gc * F + gs * 128
                nc.sync.dma_start(
                    out=out_r[t, g0 : g0 + 128, :],
                    in_=osb[t][:, gs * D : (gs + 1) * D],
                )

    # software-pipelined emission: phase1(n+1) before phase2(n)
    blocks = [(gc, cb) for gc in range(GC) for cb in range(CB)]
    osb_by_gc = {}
    for gc in range(GC):
        osb_by_gc[gc] = [
            out_pool.tile([128, GS * D], F32, tag=f"osb_{t}", name=f"osb_{gc}_{t}")
            for t in range(KS)
        ]
    state = {}
    state[blocks[0]] = phase1(blocks[0])
    for idx, blk in enumerate(blocks):
        if idx + 1 < len(blocks):
            state[blocks[idx + 1]] = phase1(blocks[idx + 1])
        phase2(blk, state.pop(blk), osb_by_gc[blk[0]])
        if blk[1] == CB - 1:
            stores(blk[0], osb_by_gc[blk[0]])


def _get_nc():
    if "nc" in _CACHE:
        return _CACHE["nc"]
    nc = bacc.Bacc("TRN2", target_bir_lowering=False, debug=False, num_devices=NCORES)
    q = nc.dram_tensor("q", [D, N], F32, kind="ExternalInput").ap()
    k = nc.dram_tensor("k", [D, N], F32, kind="ExternalInput").ap()
    v = nc.dram_tensor("v", [D, N], F32, kind="ExternalInput").ap()
    out = nc.dram_tensor("out", [N, D], F32, kind="ExternalOutput").ap()
    wsc = nc.dram_tensor("wsc", [KS * KS, 128, 96], BF16, kind="ExternalInput").ap()
    wbr = nc.dram_tensor("wbr", [KS * KS, 96, 128], BF16, kind="ExternalInput").ap()
    wd = nc.dram_tensor("wd", [96, 96], F32, kind="ExternalInput").ap()
    ident = nc.dram_tensor("ident", [128, 128], BF16, kind="ExternalInput").ap()
    with tile.TileContext(nc) as tc:
        with ExitStack() as ctx:
            _build_kernel(ctx, tc, q, k, v, out, wsc, wbr, wd, ident)
    nc.compile()
    _CACHE["nc"] = nc
    return nc


def kernel(q, k, v, head_dim, kernel_size, _trace=False, _trace_kwargs=None):
    assert int(head_dim) == HD and int(kernel_size) == KS
    q = np.asarray(q, dtype=np.float32)
    k = np.asarray(k, dtype=np.float32)
    v = np.asarray(v, dtype=np.float32)
    assert q.shape == (B, D, N)

    nc = _get_nc()
    bf = mybir.dt.np(BF16)
    wsc, wbr, wd, ident = _host_masks()
    consts = {
        "wsc": wsc.astype(bf),
        "wbr": wbr.astype(bf),
        "wd": wd,
        "ident": ident.astype(bf),
    }
    in_maps = [
        {"q": q[b], "k": k[b], "v": v[b], **consts} for b in range(B)
    ]
    res = run_bass_kernel_spmd(
        nc,
        in_maps,
        core_ids=list(range(NCORES)),
        trace=_trace,
        **(_trace_kwargs or {}),
    )
    out = np.stack([res.results[b]["out"] for b in range(B)], axis=0)
    _CACHE["last_results"] = res
    return out


if __name__ == "__main__":
    rng = np.random.default_rng(0)
    qq = rng.standard_normal((B, D, N), dtype=np.float32)
    kk = rng.standard_normal((B, D, N), dtype=np.float32)
    vv = rng.standard_normal((B, D, N), dtype=np.float32)
    o = kernel(qq, kk, vv, HD, KS)
    print("out", o.shape, o.dtype, float(np.abs(o).max()))
